# revision 55
# baseline (speedup 1.0000x reference)
"""Causal self-attention Trainium2 kernel (Bass/Tile), 8 NeuronCores.

Problem: B=2, S=2048, D=1024, H=16 heads (hd=64), fp32 in/out.
    qkv = x @ qkv_w + qkv_b ; per-head causal attention ; y = out @ out_w + out_b

Sharding (hybrid data x tensor parallel):
    8 cores = 2 batch groups x 4 head groups. Core c handles batch c//4 and
    the 4 heads [4*(c%4) .. 4*(c%4)+3]. Each core computes its partial
    out-projection y_c [S, D] in bf16; host sums the 4 partials per batch
    (fp32) and adds out_b.

Kernel strategy (v2, bf16):
    - All matmul operands bf16 (PSUM accumulation fp32): 1 cycle/row on the
      PE at any moving size (fp32r pays 4x below 256), half the DMA/SBUF.
      rel-err budget is 2e-2; bf16 end-to-end lands ~2e-3.
    - Scores computed transposed sT[k, q] = kT.T @ qT as two concurrent
      64-row PE tile matmuls (head pair), exp on ACT (the only ACT work, so
      the Exp table loads once), causal diagonal masked in-place on the
      (otherwise idle) GpSimd engine via affine_select.
    - Softmax denominator comes out of the PV matmul via a ones-column
      appended to V; 1/l via DVE reciprocal_approx_fast (single op, ~18
      good bits), broadcast across partitions with a K=1 PE matmul.
    - Attention is ACT(exp)-throughput-bound, so the emission schedule
      software-pipelines scores ahead of PV and interleaves the qkv/out
      projection matmuls as PE "filler" inside the attention loop: the PE
      never idles long enough to drop out of its boosted p-state.
"""

import os
import sys

for _p in ("/opt/trn_rl_repo", "/root/.axon_site/_ro/trn_rl_repo"):
    if os.path.isdir(_p) and _p not in sys.path:
        sys.path.insert(0, _p)

import numpy as np
import ml_dtypes
from collections import deque
from contextlib import ExitStack

import concourse.bass as bass
import concourse.tile as tile
from concourse import bacc, mybir
from concourse.bass_utils import run_bass_kernel_spmd

B, S, D = 2, 2048, 1024
H, HD = 16, 64
NCORES = 8
LOCAL_H = 4           # heads per core
P = 128
KO = D // P           # 8 contraction sub-tiles for the projections
NQ = S // 512         # 4 q-tiles of 512
NKT = S // P          # 16 k-blocks of 128
F32 = mybir.dt.float32
F32R = mybir.dt.float32r
BF16 = mybir.dt.bfloat16
AF = mybir.ActivationFunctionType
SCALE = 1.0 / np.sqrt(HD)


class Sched:
    """Round-robin over generators that emit a couple of PE matmuls per
    step; used to drip projection / out-projection work into the
    ACT-bound attention loop."""

    def __init__(self):
        self.gens = deque()

    def add(self, g):
        self.gens.append(g)

    def step(self, n):
        while n > 0 and self.gens:
            try:
                next(self.gens[0])
                n -= 1
            except StopIteration:
                self.gens.popleft()

    def flush(self):
        while self.gens:
            self.step(64)

    def pending(self):
        return len(self.gens)


def _emit(tc, nc, xT, wqk, wv, wo, bqkv, b65, onesb, y, has_qkv_bias,
          debug=None):
    with ExitStack() as ctx:
        consts = ctx.enter_context(tc.tile_pool(name="consts", bufs=1))
        persis = ctx.enter_context(tc.tile_pool(name="persist", bufs=1))
        # PSUM: scores 2 banks x2, po 1 bank x2, filler (proj/outproj/rb)
        # 1 bank x2  -> 8 banks total
        pss = ctx.enter_context(tc.tile_pool(name="pss", bufs=2, space="PSUM"))
        psp = ctx.enter_context(tc.tile_pool(name="psp", bufs=2, space="PSUM"))
        psf = ctx.enter_context(tc.tile_pool(name="psf", bufs=2, space="PSUM"))
        work = ctx.enter_context(tc.tile_pool(name="work", bufs=4))
        ypool = ctx.enter_context(tc.tile_pool(name="yp", bufs=3))
        small = ctx.enter_context(tc.tile_pool(name="small", bufs=4))

        # ---- constant loads ----
        b65_sb = consts.tile([1, 260], BF16)
        nc.scalar.dma_start(b65_sb[:], b65[None, :])
        ones_bf = consts.tile([1, P], BF16)
        nc.scalar.dma_start(ones_bf[:], onesb[None, :])
        # lower-triangle keep-mask for diagonal 128x128 score blocks
        mask128 = consts.tile([P, P], BF16)
        nc.scalar.dma_start(mask128[:], onesb[None, :].to_broadcast((P, P)))
        nc.gpsimd.affine_select(
            out=mask128[:], in_=mask128[:], pattern=[[1, P]],
            compare_op=mybir.AluOpType.is_ge, fill=0.0, base=0,
            channel_multiplier=-1,
        )
        if has_qkv_bias:
            bqk_sb = consts.tile([P, 4], F32)
            nc.scalar.dma_start(bqk_sb[:], bqkv[0:512].rearrange("(m p) -> p m", p=P))

        # x/wqk pairs round-robin over the three DMA-capable queues in ko
        # order (the projection ko-chains consume them in this order)
        x_sb, wqk_t, wv_t = [], [], []
        QS = [nc.sync, nc.gpsimd, nc.scalar]
        for ko in range(KO):
            w = consts.tile([P, 512], BF16, name=f"wqk{ko}")
            QS[(2 * ko + 1) % 3].dma_start(w[:], wqk[ko * P:(ko + 1) * P, :])
            wqk_t.append(w)
            t = consts.tile([P, S], BF16, name=f"x{ko}")
            QS[(2 * ko) % 3].dma_start(t[:], xT[ko * P:(ko + 1) * P, :])
            x_sb.append(t)
        NV = 260 if has_qkv_bias else 256
        for ko in range(KO):
            w = consts.tile([P, NV], BF16, name=f"wv{ko}")
            QS[ko % 3].dma_start(w[:], wv[ko * P:(ko + 1) * P, :])
            wv_t.append(w)
        wo_sb = consts.tile([P, 2, D], BF16)
        nc.scalar.dma_start(wo_sb[:], wo.rearrange("(ks p) n -> p ks n", p=P))

        # persistent activations
        qkT = persis.tile([P, 4, S], BF16)       # m 0,1: qT(h0..h3); 2,3: kT
        v_all = persis.tile([P, NKT, LOCAL_H, 65], BF16)
        outT = persis.tile([P, 2, S], BF16)      # attention out^T (out lhsT)
        if not has_qkv_bias:
            # l-accumulator column of V (softmax denominator via PV matmul)
            nc.vector.memset(v_all[:, :, :, 64:65], 1.0)

        # ---- projection groups (generators; emitted via Sched) ----
        # NOTE: each filler group is emitted atomically (one yield per
        # group). A psf tag-"f" tile must never be held across other
        # tag-"f" allocations (the 2-slot ring would hand its bank to a
        # later alloc mid-accumulation).
        def qk_group(m, n):
            ps = psf.tile([P, 512], F32, tag="f", name=f"qk{m}{n}")
            for ko in range(KO):
                nc.tensor.matmul(
                    ps[:],
                    (wqk_t[ko][:, m * P:(m + 1) * P]),
                    (x_sb[ko][:, n * 512:(n + 1) * 512]),
                    start=(ko == 0), stop=(ko == KO - 1),
                )
            dst = qkT[:, m, n * 512:(n + 1) * 512]
            if has_qkv_bias:
                nc.scalar.activation(dst, ps[:], AF.Identity,
                                     bias=bqk_sb[:, m:m + 1])
            else:
                nc.vector.tensor_copy(dst, ps[:])
            yield

        def v_group(mt):
            ps = psf.tile([P, 512], F32, tag="f", name=f"vp{mt}")
            pv = ps[:, 0:NV]
            for ko in range(KO):
                nc.tensor.matmul(
                    pv,
                    (x_sb[ko][:, mt * P:(mt + 1) * P]),
                    (wv_t[ko][:]),
                    start=(ko == 0), stop=(not has_qkv_bias and ko == KO - 1),
                )
            if has_qkv_bias:
                nc.tensor.matmul(pv, (ones_bf[:1, :]), (b65_sb[:1, :]),
                                 start=False, stop=True)
                nc.vector.tensor_copy(
                    v_all[:, mt, :, :],
                    pv.rearrange("p (h d) -> p h d", h=LOCAL_H),
                )
            else:
                nc.vector.tensor_copy(
                    v_all[:, mt, :, 0:64],
                    pv.rearrange("p (h d) -> p h d", h=LOCAL_H),
                )
            yield

        def oproj_unit(mt, n2):
            ps = psf.tile([P, 512], F32, tag="f", name=f"op{mt}{n2}")
            for ks in range(2):
                nc.tensor.matmul(
                    ps[:],
                    (outT[:, ks, mt * P:(mt + 1) * P]),
                    (wo_sb[:, ks, n2 * 512:(n2 + 1) * 512]),
                    start=(ks == 0), stop=(ks == 1),
                )
            yt = ypool.tile([P, 512], BF16, tag="y")
            nc.vector.tensor_copy(yt[:], ps[:])
            eng = nc.gpsimd if (mt + n2) % 2 == 0 else nc.sync
            eng.dma_start(
                y[mt * P:(mt + 1) * P, n2 * 512:(n2 + 1) * 512], yt[:])
            yield

        def proj_groups_for(jq):
            gens = []
            for m in (2, 0):           # kT then qT for this n-range
                gens.append(qk_group(m, jq))
                gens.append(qk_group(m + 1, jq))
            for mt in range(4 * jq, 4 * jq + 4):
                gens.append(v_group(mt))
            return gens

        sched = Sched()

        # prime: only what attention(jq=0, hp=0) needs; the hp=1 projections
        # (kT m3 / qT m1) run as the first fillers inside the hp=0 window
        prime = [qk_group(2, 0), qk_group(0, 0)] + \
                [v_group(mt) for mt in range(4)]
        for g in prime:
            sched.add(g)
        sched.flush()
        sched.add(qk_group(3, 0))
        sched.add(qk_group(1, 0))

        # ---- attention (jq outer), with pipelined scores and fillers ----
        def emit_scores(jq, hp, kt):
            rel = kt - 4 * jq
            f0 = 128 * rel if rel > 0 else 0
            ps = pss.tile([P, 2, 512], F32, tag="s", name=f"s{jq}{hp}{kt}")
            for i in range(2):
                poff = 64 * i
                nc.tensor.matmul(
                    ps[:, i, f0:512],
                    (qkT[poff:poff + 64, 2 + hp, kt * P:(kt + 1) * P]),
                    (qkT[poff:poff + 64, hp,
                         jq * 512 + f0:(jq + 1) * 512]),
                    start=True, stop=True,
                )
            return ps, f0, rel

        for jq in range(NQ):
            # everything still queued (this jq's projections) must be fully
            # emitted before attention(jq) reads qkT/v_all
            if jq >= 1:
                sched.flush()
            # queue fillers: projections for jq+1, out-proj for jq-1
            if jq + 1 < NQ:
                for g in proj_groups_for(jq + 1):
                    sched.add(g)
            if jq >= 1:
                for mt in range(4 * (jq - 1), 4 * (jq - 1) + 4):
                    for n2 in range(2):
                        sched.add(oproj_unit(mt, n2))

            last_kt = 4 * jq + 3
            nslots = 2 * (last_kt + 1)
            slot = 0
            for hp in range(2):
                if jq == 0 and hp == 1:
                    # the kT m3 / qT m1 fillers are FIFO positions 1-2;
                    # hp=1 scores read them, so force them out now
                    sched.step(2)
                po = [psp.tile([65, 512], F32, tag="po", name=f"po{hp}{i_}")
                      for i_ in range(2)]
                sc = emit_scores(jq, hp, 0)
                for kt in range(last_kt + 1):
                    ps, f0, rel = sc
                    et = work.tile([P, 2, 512], BF16, tag="e")
                    nc.scalar.activation(et[:, :, f0:512], ps[:, :, f0:512],
                                         AF.Exp, scale=float(SCALE))
                    if rel >= 0:   # mask the 128-wide triangle
                        for i in range(2):
                            nc.vector.tensor_tensor(
                                et[:, i, f0:f0 + 128], et[:, i, f0:f0 + 128],
                                mask128[:], mybir.AluOpType.mult)
                    if kt < last_kt:
                        sc = emit_scores(jq, hp, kt + 1)
                    # drip filler PE work while ACT chews on exp; pace so
                    # the queue drains just as the window ends
                    rem = nslots - slot
                    slot += 1
                    pend = max(0, sched.pending() - 2)   # hold 2 for norm
                    pace = -(-pend // rem) if rem > 0 else 64
                    sched.step(pace)
                    for i in range(2):
                        lh = 2 * hp + i
                        nc.tensor.matmul(
                            po[i][:, f0:512],
                            (v_all[:, kt, lh, :]),
                            (et[:, i, f0:512]),
                            start=(kt == 0), stop=(kt == last_kt),
                        )
                # normalization: rr = 1/l (reciprocal_approx_fast needs
                # base-partition-0 operands, so stage the l row to a base-0
                # tile first), broadcast across partitions on the otherwise
                # idle GpSimd engine, then scale po directly out of PSUM.
                for i in range(2):
                    lr = small.tile([1, 512], F32, tag=f"lr{i}")
                    nc.vector.tensor_copy(lr[:, :], po[i][64:65, :])
                    rr = small.tile([1, 512], F32, tag=f"rr{i}")
                    nc.vector.reciprocal_approx_fast(rr[:, :], lr[:, :])
                    rb = work.tile([64, 512], F32, tag=f"rb{i}")
                    nc.gpsimd.partition_broadcast(rb[:], rr[:1, :])
                    sched.step(1)   # keep the PE fed during the 1/l chain
                    nc.vector.tensor_tensor(
                        outT[64 * i:64 * i + 64, hp, jq * 512:(jq + 1) * 512],
                        po[i][0:64, :], rb[:], mybir.AluOpType.mult,
                    )

        # tail: last out-projection (+ anything left over)
        for mt in range(4 * (NQ - 1), 4 * (NQ - 1) + 4):
            for n2 in range(2):
                sched.add(oproj_unit(mt, n2))
        sched.flush()
        if debug is not None:
            dq, dv, do = debug
            nc.sync.dma_start(dq[:], qkT[:])
            nc.sync.dma_start(dv[:], v_all[:])
            nc.sync.dma_start(do[:], outT[:])


def build_nc(has_qkv_bias):
    nc = bacc.Bacc("TRN2", target_bir_lowering=False, debug=False,
                   num_devices=NCORES)
    xT = nc.dram_tensor("xT", [D, S], BF16, kind="ExternalInput")
    wqk = nc.dram_tensor("wqk", [D, 512], BF16, kind="ExternalInput")
    wv = nc.dram_tensor("wv", [D, 260 if has_qkv_bias else 256], BF16,
                        kind="ExternalInput")
    wo = nc.dram_tensor("wo", [2 * P, D], BF16, kind="ExternalInput")
    bqkv = nc.dram_tensor("bqkv", [768], F32, kind="ExternalInput")
    b65 = nc.dram_tensor("b65", [260], BF16, kind="ExternalInput")
    onesb = nc.dram_tensor("onesb", [P], BF16, kind="ExternalInput")
    y = nc.dram_tensor("y", [S, D], BF16, kind="ExternalOutput")
    with tile.TileContext(nc) as tc:
        _emit(tc, nc, xT.ap(), wqk.ap(), wv.ap(), wo.ap(), bqkv.ap(), b65.ap(),
              onesb.ap(), y.ap(), has_qkv_bias)
    nc.compile()
    return nc


_NC_CACHE = {}


def _get_nc(has_qkv_bias):
    key = bool(has_qkv_bias)
    if key not in _NC_CACHE:
        _NC_CACHE[key] = build_nc(key)
    return _NC_CACHE[key]


def make_in_maps(x, qkv_w, qkv_b, out_w, has_qkv_bias=None):
    """Per-core host-side sharding. Core c: batch c//4, heads 4*(c%4)..+3."""
    bf = ml_dtypes.bfloat16
    if has_qkv_bias is None:
        has_qkv_bias = bool(np.any(qkv_b))
    in_maps = []
    xTs = [np.ascontiguousarray(x[b].T).astype(bf) for b in range(B)]
    for c in range(NCORES):
        b = c // (NCORES // B)
        g = c % (NCORES // B)
        h0 = LOCAL_H * g
        cols = slice(h0 * HD, (h0 + LOCAL_H) * HD)
        wq = qkv_w[:, cols]
        wk = qkv_w[:, D:][:, cols]
        wv_ = qkv_w[:, 2 * D:][:, cols]
        bq = qkv_b[cols]
        bk = qkv_b[D:][cols]
        bv = qkv_b[2 * D:][cols]
        if has_qkv_bias:
            wv_pad = np.zeros((D, LOCAL_H, 65), np.float32)
            wv_pad[:, :, :64] = wv_.reshape(D, LOCAL_H, HD)
            wv_packed = wv_pad.reshape(D, LOCAL_H * 65)
        else:
            wv_packed = wv_
        b65_arr = np.zeros((LOCAL_H, 65), np.float32)
        b65_arr[:, :64] = bv.reshape(LOCAL_H, HD)
        b65_arr[:, 64] = 1.0
        in_maps.append({
            "xT": xTs[b],
            "wqk": np.concatenate([wq, wk], axis=1).astype(bf),
            "wv": np.ascontiguousarray(wv_packed).astype(bf),
            "wo": np.ascontiguousarray(out_w[cols, :]).astype(bf),
            "bqkv": np.ascontiguousarray(np.concatenate([bq, bk, bv])),
            "b65": b65_arr.reshape(-1).astype(bf),
            "onesb": np.ones(P, bf),
        })
    return in_maps


def _ensure_ntff_hook():
    """Provide antenv.axon_hooks (missing in this image) so trace=True works."""
    try:
        from antenv.axon_hooks import get_axon_ntff_profile_hook  # noqa: F401
        return
    except ImportError:
        pass
    import types
    import antenv
    mod = types.ModuleType("antenv.axon_hooks")
    holder = {"hook": None}
    mod.set_axon_ntff_profile_hook = lambda h: holder.__setitem__("hook", h)
    mod.get_axon_ntff_profile_hook = lambda: holder["hook"]
    sys.modules["antenv.axon_hooks"] = mod
    antenv.axon_hooks = mod
    try:
        from trn_agent_boot.trn_boot import _ntff_profile_via_ctypes
        so = "/opt/axon/libaxon_pjrt.so"
        if os.path.exists(so):
            mod.set_axon_ntff_profile_hook(_ntff_profile_via_ctypes(so))
    except Exception:
        pass


def kernel(x, qkv_w, qkv_b, out_w, out_b, _trace=False):
    if _trace:
        _ensure_ntff_hook()
    x = np.asarray(x, dtype=np.float32)
    qkv_w = np.asarray(qkv_w, dtype=np.float32)
    qkv_b = np.asarray(qkv_b, dtype=np.float32)
    out_w = np.asarray(out_w, dtype=np.float32)
    out_b = np.asarray(out_b, dtype=np.float32)

    has_qkv_bias = bool(np.any(qkv_b))
    nc = _get_nc(has_qkv_bias)
    in_maps = make_in_maps(x, qkv_w, qkv_b, out_w, has_qkv_bias)
    res = run_bass_kernel_spmd(nc, in_maps, core_ids=list(range(NCORES)),
                               trace=_trace)
    y = np.zeros((B, S, D), dtype=np.float32)
    for c in range(NCORES):
        y[c // (NCORES // B)] += np.asarray(res.results[c]["y"],
                                            dtype=np.float32)
    y += out_b
    if _trace:
        kernel.last_results = res
    return y
